# revision 15
# baseline (speedup 1.0000x reference)
"""Trainium2 Bass kernel for nn_MemoryReader (scatter_memory).

Math (per batch b):
  affinity[t,q] = (-||mk_t||^2 + 2 mk_t.qk_q) / sqrt(CK)        [THW, HW]
  affinity *= mult[t]          (mult from mask/argsort/RNG scatter)
  w = softmax(affinity, axis=t)
  mem[c,q] = sum_t mv[c,t] w[t,q];  out = concat([mem, qv], ch)

Key identity: support = affinity.mean(axis=q) is affine in mean(qk), so the
whole _reallocate scatter collapses to a tiny host-side computation of
mult[B,THW].  Device does the heavy part:
  S = mk^T qk'            (qk' = qk/4, so S = (2 mk.qk)/8)
  p = exp(S*scl_t + bia_t)  with scl_t = mult_t, bia_t = -||mk_t||^2/8*mult_t - C
  Z = sum_t p ;  mem = (mvT^T p) / Z
No max-subtraction needed: C is a host-computed global bound keeping exp in
range; softmax is shift-invariant so the result is unchanged.

Sharding: 8 cores = 4 batches x 2 query-halves (HW=1024 -> 512 per core).
Softmax axis (t) stays on-core -> no collectives.
"""

import math
import os
import sys

import numpy as np

B, CK, CV, T, H, W = 4, 64, 512, 8, 32, 32
THW, HW = T * H * W, H * W          # 8192, 1024
QH = HW // 2                        # 512 queries per core
NT = THW // 128                     # 64 token tiles
NC_CORES = 8

_PROGRAM = None


def _ensure_path():
    for p in ("/opt/trn_rl_repo",):
        if p not in sys.path and os.path.isdir(p):
            sys.path.insert(0, p)


def _build_program():
    """One SPMD Bass program; per-core data differs via in_maps."""
    _ensure_path()
    from contextlib import ExitStack

    import concourse.bacc as bacc
    import concourse.mybir as mybir
    import concourse.tile as tile

    f16 = mybir.dt.float16
    bf16 = mybir.dt.bfloat16
    f32 = mybir.dt.float32

    nc = bacc.Bacc(None, debug=False)
    mk_d = nc.declare_dram_parameter("mk", [CK, THW], f16, isOutput=False)
    qk_d = nc.declare_dram_parameter("qkp", [CK, QH], f16, isOutput=False)
    mvt_d = nc.declare_dram_parameter("mvt", [THW, CV], bf16, isOutput=False)
    sbv_d = nc.declare_dram_parameter("sbv", [128, 2 * NT], f32, isOutput=False)
    out_d = nc.declare_dram_parameter("out", [4, 128, QH], f32, isOutput=True)

    with tile.TileContext(nc) as tc, ExitStack() as ctx:
        const = ctx.enter_context(tc.tile_pool(name="const", bufs=1))
        mv_pool = ctx.enter_context(tc.tile_pool(name="mv", bufs=6))
        p_pool = ctx.enter_context(tc.tile_pool(name="p", bufs=6))
        o_pool = ctx.enter_context(tc.tile_pool(name="o", bufs=1))
        s_psum = ctx.enter_context(tc.tile_pool(name="spsum", bufs=2, space="PSUM"))
        o_psum = ctx.enter_context(tc.tile_pool(name="opsum", bufs=1, space="PSUM"))
        r_psum = ctx.enter_context(tc.tile_pool(name="rpsum", bufs=1, space="PSUM"))

        mk_sb = const.tile([CK, THW], f16, tag="mk")
        nc.sync.dma_start(out=mk_sb[:], in_=mk_d[:])
        qk_sb = const.tile([CK, QH], f16, tag="qk")
        nc.sync.dma_start(out=qk_sb[:], in_=qk_d[:])
        sbv_sb = const.tile([128, 2 * NT], f32, tag="sbv")
        nc.sync.dma_start(out=sbv_sb[:], in_=sbv_d[:])

        # ACT warm-up: consume the sbv DMA sem on the ACT engine before the
        # loop so the first Exp (which also carries the ACT table-load pseudo)
        # needs only the PE wait — ACT has limited sync-wait slots.
        warm = const.tile([128, 1], f32, tag="warm")
        nc.scalar.copy(out=warm[:], in_=sbv_sb[:, 0:1])

        zacc = const.tile([128, QH], f32, tag="zacc")
        nc.vector.memset(zacc[:], 0.0)
        ones_sb = const.tile([128, 128], f32, tag="ones")
        nc.vector.memset(ones_sb[:], 1.0)

        out_ps = [
            o_psum.tile([128, QH], f32, tag=f"out{c}", name=f"out_ps{c}")
            for c in range(4)
        ]

        for ti in range(NT):
            mv_sb = mv_pool.tile([128, CV], bf16, tag="mvt")
            nc.sync.dma_start(out=mv_sb[:], in_=mvt_d[ti * 128:(ti + 1) * 128, :])

            sp = s_psum.tile([128, QH], f32, tag="s")
            nc.tensor.matmul(
                sp[:], lhsT=mk_sb[:, ti * 128:(ti + 1) * 128], rhs=qk_sb[:],
                start=True, stop=True,
            )
            p_sb = p_pool.tile([128, QH], bf16, tag="p")
            nc.scalar.activation(
                p_sb[:], sp[:], mybir.ActivationFunctionType.Exp,
                bias=sbv_sb[:, NT + ti:NT + ti + 1], scale=sbv_sb[:, ti:ti + 1],
            )
            for c in range(4):
                nc.tensor.matmul(
                    out_ps[c][:], lhsT=mv_sb[:, c * 128:(c + 1) * 128], rhs=p_sb[:],
                    start=(ti == 0), stop=(ti == NT - 1),
                )
            nc.vector.tensor_add(zacc[:], zacc[:], p_sb[:])

        # Zb[p,q] = sum over 128 partitions of zacc, broadcast to all 128
        # output partitions in one matmul: ones[128,128]^T @ zacc
        zb = r_psum.tile([128, QH], f32, tag="zb")
        nc.tensor.matmul(zb[:], lhsT=ones_sb[:], rhs=zacc[:], start=True, stop=True)
        rb_sb = const.tile([128, QH], f32, tag="rbsb")
        nc.vector.reciprocal(rb_sb[:], zb[:])

        for c in range(4):
            o_sb = o_pool.tile([128, QH], f32, tag=f"osb{c}")
            nc.vector.tensor_mul(o_sb[:], out_ps[c][:], rb_sb[:])
            nc.sync.dma_start(out=out_d[c], in_=o_sb[:])

    if not nc.is_finalized():
        nc.finalize()
    return nc


def _threefry_u():
    """The reference's fixed RNG draws: uniform(key(1234), (B, THW//2)) f32."""
    import jax

    with jax.default_device(jax.devices("cpu")[0]):
        u = jax.random.uniform(
            jax.random.key(1234), (B, THW // 2), dtype=np.float32
        )
        return np.asarray(u)


def _mult_from_support(support, mask):
    """Exact numpy mirror of reference._reallocate's mult computation."""
    p = np.float32(0.5)
    m = mask.reshape(B, -1)
    masked = np.where(m > 0.5, support, np.float32(0.0)).astype(np.float32)
    sorted_idx = np.argsort(-masked, axis=1, kind="stable")
    k = (masked != 0).sum(axis=1)
    kf = k.astype(np.float32)
    M = THW // 2
    u = _threefry_u()
    idx = np.floor(u * kf[:, None]).astype(np.int32)
    n_sel = np.floor(kf * p).astype(np.int32)
    valid = np.arange(M, dtype=np.int32)[None, :] < n_sel[:, None]
    w_sel = (idx + 1).astype(np.float32)
    s = np.where(valid, w_sel, np.float32(0.0)).sum(axis=1, dtype=np.float32)
    s = np.where(s > 0, s, np.float32(1.0))[:, None]
    w_i = w_sel * kf[:, None] / s
    rows = np.take_along_axis(sorted_idx, idx, axis=1)
    mult = np.ones((B, THW), np.float32)
    bidx = np.broadcast_to(np.arange(B)[:, None], rows.shape)
    mult[bidx[valid], rows[valid]] = w_i[valid]
    return mult


def kernel(mk, qk, mv, qv, mask):
    global _PROGRAM
    _ensure_path()
    from concourse import bass_utils

    mk = np.asarray(mk, np.float32)
    qk = np.asarray(qk, np.float32)
    mv = np.asarray(mv, np.float32)
    qv = np.asarray(qv, np.float32)
    mask = np.asarray(mask, np.float32)

    rs = np.float32(1.0 / math.sqrt(CK))          # 1/8
    mk_f = mk.reshape(B, CK, THW)
    qk_f = qk.reshape(B, CK, HW)
    sumsq = np.einsum("bct,bct->bt", mk_f, mk_f)            # [B, THW]
    qbar = qk_f.mean(axis=2)                                 # [B, CK]
    support = (-sumsq + 2.0 * np.einsum("bct,bc->bt", mk_f, qbar)) * rs
    mult = _mult_from_support(support.astype(np.float32), mask)

    bias0 = -sumsq * rs                                      # [B, THW]
    # global per-batch shift C: bound on max affinity (Cauchy-Schwarz), keeps
    # exp() in fp32 range; softmax is shift-invariant so result is unchanged
    mk_norm = np.sqrt(sumsq)
    qk_nmax = np.sqrt(np.einsum("bcq,bcq->bq", qk_f, qk_f)).max(axis=1)  # [B]
    ub = (mult * (mk_norm * qk_nmax[:, None] * 0.25 + bias0)).max(axis=1)
    Cshift = np.maximum(0.0, ub - 50.0).astype(np.float32)   # usually 0

    scl = mult                                               # [B, THW]
    bia = (bias0 * mult - Cshift[:, None]).astype(np.float32)

    mk16 = mk_f.astype(np.float16)
    qkp16 = (qk_f * 0.25).astype(np.float16)                 # folds the 2/8
    import ml_dtypes
    mvt16 = np.ascontiguousarray(
        mv.reshape(B, CV, THW).transpose(0, 2, 1)
    ).astype(ml_dtypes.bfloat16)                             # [B, THW, CV]

    def vec_layout(v):                                       # [THW] -> [128, NT]
        return np.ascontiguousarray(v.reshape(NT, 128).T)

    in_maps = []
    for i in range(NC_CORES):
        b, qh = i // 2, i % 2
        in_maps.append({
            "mk": np.ascontiguousarray(mk16[b]),
            "qkp": np.ascontiguousarray(qkp16[b, :, qh * QH:(qh + 1) * QH]),
            "mvt": mvt16[b],
            "sbv": np.ascontiguousarray(
                np.concatenate([vec_layout(scl[b]), vec_layout(bia[b])], axis=1)
            ),
        })

    if _PROGRAM is None:
        _PROGRAM = _build_program()

    global _LAST_IN_MAPS
    _LAST_IN_MAPS = in_maps
    res = bass_utils.run_bass_kernel_spmd(
        _PROGRAM, in_maps, core_ids=list(range(NC_CORES))
    ).results

    out = np.empty((B, 2 * CV, H, W), np.float32)
    mem = out[:, :CV].reshape(B, CV, HW)
    for i in range(NC_CORES):
        b, qh = i // 2, i % 2
        mem[b, :, qh * QH:(qh + 1) * QH] = res[i]["out"].reshape(CV, QH)
    out[:, CV:] = qv
    return out
